# revision 53
# baseline (speedup 1.0000x reference)
"""TRN2 Bass kernel for nn_Aij (GAT-style dense attention coefficients).

Math (H=1 collapses the reference):
    s[b,i] = (encode[b,i,:] @ W) @ v_self      (scalar per node)
    n[b,j] = (encode[b,j,:] @ W) @ v_neigh     (scalar per node)
    out[b,i,j] = softmax_j( leaky_relu(s[b,i] + n[b,j], 0.2) )

Output is [8, 2048, 2048] f32 = 128 MiB -> memory-bound on the output store.

Sharding: data-parallel over batch; core b computes batch b (16 MiB store/core).

Device-side structure per core (16 row tiles of 128 x 2048):
  - PE   : t02[i,j] = 0.2*(s_i + n_j) via K=6 bf16 matmul into PSUM. bf16
           runs 4x faster than fp32 on the PE; fp32-equivalent precision
           comes from 3-term bf16 splits of 0.2s and 0.2n:
           lhsT rows [q_hi,q_lo,q_lo2,1,1,1], rhs rows [1,1,1,p_hi,p_lo,p_lo2].
  - DVE  : ONE fused op per tile: leaky_relu(t) = (nb + s_i) max PSUM_t02
           via scalar_tensor_tensor (t recomputed exactly in fp32; 0.2t from
           the PE; only one PSUM operand, which is the HW limit).
  - ACT  : out = Exp(L + bias_i), bias_i = -ln(rowsum_i) per-partition AP.
           Tile 0 computes unscaled t on the PE instead and runs its lrelu as
           ACT Prelu(alpha=0.2) straight from PSUM in column halves, so the
           first stores issue before the n-broadcast load lands; tile 1 runs
           its stt/exp in halves behind the two nb load chunks. Steady state
           is store-DMA-bound.
  - DMA  : 1 MiB store per row tile, streamed back-to-back at the HBM
           per-core limit (cost model: zero inter-store gaps after tile 0).

The softmax denominator rowsum_i = sum_j exp(lrelu(s_i+n_j)) depends only on
the O(N) vectors s, n: with n sorted, the sum splits at the lrelu knee into
prefix/suffix sums, so it is computed exactly (f64) on the host in O(N log N)
and folded into the per-partition Exp bias. This removes the normalization
pass entirely; all O(N^2) work runs on device.
"""

import numpy as np
from ml_dtypes import bfloat16

B, N, F = 8, 2048, 64
P = 128  # partitions
NT = N // P  # 16 row tiles
ACT_LRELU_TILES = frozenset((0,))  # startup tiles: lrelu on ACT (no nb dep)

_compiled = None


def _build(reps=1):
    from contextlib import ExitStack

    import concourse.bacc as bacc
    import concourse.mybir as mybir
    import concourse.tile as tile

    F32 = mybir.dt.float32
    BF16 = mybir.dt.bfloat16

    nc = bacc.Bacc("TRN2", target_bir_lowering=False)

    # K=6 bf16 matmuls at fp32-equivalent precision via 3-term bf16 splits
    # (bf16 PE runs 4x faster than fp32):
    #   mm_pack  -> t   = s_i + n_j         (tile 0 only, feeds ACT Prelu)
    #   mm2_pack -> t02 = 0.2*(s_i + n_j)   (tiles 1+, feeds the DVE stt)
    # each [6, 2N]: cols 0:N = rhs rows; cols N:2N = lhsT rows
    mm_pack = nc.dram_tensor("mm_pack", [6, 2 * N], BF16, kind="ExternalInput")
    mm2_pack = nc.dram_tensor("mm2_pack", [6, 2 * N], BF16, kind="ExternalInput")
    # spack: cols 0:NT = s cols, NT:2*NT = bias cols (tiny, loaded first)
    spack = nc.dram_tensor("spack", [P, 2 * NT], F32, kind="ExternalInput")
    # nbpack: n broadcast to all partitions
    nbpack = nc.dram_tensor("nbpack", [P, N], F32, kind="ExternalInput")
    out = nc.dram_tensor("out", [N, N], F32, kind="ExternalOutput")

    with tile.TileContext(nc) as tc, ExitStack() as ctx:
        singles = ctx.enter_context(tc.tile_pool(name="singles", bufs=1))
        psum = ctx.enter_context(tc.tile_pool(name="psum", bufs=2, space="PSUM"))
        lp = ctx.enter_context(tc.tile_pool(name="lp", bufs=4))
        outp = ctx.enter_context(tc.tile_pool(name="outp", bufs=4))

        sp_sb = singles.tile([P, 2 * NT], F32)
        nc.scalar.dma_start(out=sp_sb, in_=spack[:, :])
        mm_sb = singles.tile([6, 2 * N], BF16)
        nc.sync.dma_start(out=mm_sb, in_=mm_pack[:, :])
        mm2_sb = singles.tile([6, 2 * N], BF16)
        nc.sync.dma_start(out=mm2_sb, in_=mm2_pack[:, :])
        nb = singles.tile([P, N], F32)
        nc.sync.dma_start(out=nb[:, 0:N // 2], in_=nbpack[:, 0:N // 2])
        nc.sync.dma_start(out=nb[:, N // 2 :], in_=nbpack[:, N // 2 :])


        H = N // 2
        prev_act = None
        for _rep, k in [(r, kk) for r in range(reps) for kk in range(NT)]:
          if True:
            src_sb = mm_sb if k in ACT_LRELU_TILES else mm2_sb
            lhsT = src_sb[0:6, N + P * k : N + P * (k + 1)]
            psum_t = psum.tile([P, N], F32)
            for c in range(4):
                nc.tensor.matmul(
                    psum_t[:, 512 * c : 512 * (c + 1)],
                    lhsT,
                    src_sb[0:6, 512 * c : 512 * (c + 1)],
                    start=True,
                    stop=True,
                )

            if k in ACT_LRELU_TILES:
                # startup tile: leaky-relu on ACT straight from PSUM (no nb
                # dep), in halves so the first store issues earliest
                for h in range(2):
                    lt_a = lp.tile([P, H], F32, tag="lt_h")
                    i1 = nc.scalar.activation(
                        out=lt_a, in_=psum_t[:, H * h : H * (h + 1)],
                        func=mybir.ActivationFunctionType.Prelu,
                        bias=0.0, scale=1.0, alpha=0.2,
                    )
                    if prev_act is not None:
                        tile.add_dep_helper(i1.ins, prev_act.ins, reason="act order")
                    ot_a = outp.tile([P, H], F32, tag="ot_h")
                    i2 = nc.scalar.activation(
                        out=ot_a, in_=lt_a,
                        func=mybir.ActivationFunctionType.Exp,
                        bias=sp_sb[:, NT + k : NT + k + 1],
                        scale=1.0,
                    )
                    tile.add_dep_helper(i2.ins, i1.ins, reason="act order")
                    prev_act = i2
                    nc.sync.dma_start(
                        out=out[P * k : P * (k + 1), H * h : H * (h + 1)],
                        in_=ot_a,
                    )
                continue

            # single fused DVE op: leaky_relu(t) = (nb + s_i) max psum_t02
            # (t recomputed exactly in fp32 by the stt; 0.2t from the PE);
            # tile 1 runs in halves so it starts after the first nb chunk
            lt = lp.tile([P, N], F32, tag="lt")
            hs = 2 if k == 1 else 1
            for hq in range(hs):
                w = N // hs
                nc.vector.scalar_tensor_tensor(
                    out=lt[:, w * hq : w * (hq + 1)],
                    in0=nb[:, w * hq : w * (hq + 1)],
                    scalar=sp_sb[:, k : k + 1],
                    in1=psum_t[:, w * hq : w * (hq + 1)],
                    op0=mybir.AluOpType.add,
                    op1=mybir.AluOpType.max,
                )

            if k == 1:
                for hq in range(2):
                    ot_h = outp.tile([P, H], F32, tag="ot_h")
                    nc.scalar.activation(
                        out=ot_h,
                        in_=lt[:, H * hq : H * (hq + 1)],
                        func=mybir.ActivationFunctionType.Exp,
                        bias=sp_sb[:, NT + k : NT + k + 1],
                        scale=1.0,
                    )
                    nc.sync.dma_start(
                        out=out[P * k : P * (k + 1), H * hq : H * (hq + 1)],
                        in_=ot_h,
                    )
            else:
                ot = outp.tile([P, N], F32, tag="ot")
                nc.scalar.activation(
                    out=ot,
                    in_=lt,
                    func=mybir.ActivationFunctionType.Exp,
                    bias=sp_sb[:, NT + k : NT + k + 1],
                    scale=1.0,
                )
                nc.sync.dma_start(out=out[P * k : P * (k + 1), :], in_=ot)

    nc.compile()
    return nc


def _get_compiled(reps=1):
    global _compiled
    if _compiled is None:
        _compiled = {}
    if reps not in _compiled:
        _compiled[reps] = _build(reps)
    return _compiled[reps]


def _host_prep(encode, kernel, attn_kernel_self, attn_kernel_neighs):
    """Per-batch scalars s, n and exact row-sum biases; device input packing."""
    enc = np.asarray(encode, np.float32)
    W = np.asarray(kernel, np.float32)[:, 0, :]
    v_s = np.asarray(attn_kernel_self, np.float32)[:, 0, 0]
    v_n = np.asarray(attn_kernel_neighs, np.float32)[:, 0, 0]

    # same association order as the reference: h = enc @ W, then h @ v
    h = enc.reshape(B * N, F) @ W
    s_all = (h @ v_s).reshape(B, N).astype(np.float32)
    n_all = (h @ v_n).reshape(B, N).astype(np.float32)

    mm_packs, vec_packs = [], []
    for b in range(B):
        s, n = s_all[b], n_all[b]

        # exact rowsums: S_i = sum_j exp(lrelu(s_i + n_j)) via sorted split
        s64 = s.astype(np.float64)
        n64 = np.sort(n.astype(np.float64))
        suf = np.concatenate([np.cumsum(np.exp(n64)[::-1])[::-1], [0.0]])
        pre = np.concatenate([[0.0], np.cumsum(np.exp(0.2 * n64))])
        idx = np.searchsorted(n64, -s64, side="right")
        S = np.exp(s64) * suf[idx] + np.exp(0.2 * s64) * pre[idx]
        bias = (-np.log(S)).astype(np.float32)

        def split3(x):
            hi = x.astype(bfloat16)
            lo = (x - hi.astype(np.float32)).astype(bfloat16)
            lo2 = (x - hi.astype(np.float32) - lo.astype(np.float32)).astype(bfloat16)
            return hi, lo, lo2

        s_sp, n_sp = split3(s), split3(n)
        s02_sp = split3((0.2 * s.astype(np.float64)).astype(np.float32))
        n02_sp = split3((0.2 * n.astype(np.float64)).astype(np.float32))
        mm_pack = np.zeros((6, 2 * N), bfloat16)
        mm2_pack = np.zeros((6, 2 * N), bfloat16)
        for r in range(3):
            mm_pack[r, 0:N] = bfloat16(1.0)
            mm_pack[r, N:] = s_sp[r]
            mm_pack[3 + r, 0:N] = n_sp[r]
            mm_pack[3 + r, N:] = bfloat16(1.0)
            mm2_pack[r, 0:N] = bfloat16(1.0)
            mm2_pack[r, N:] = s02_sp[r]
            mm2_pack[3 + r, 0:N] = n02_sp[r]
            mm2_pack[3 + r, N:] = bfloat16(1.0)

        spack = np.empty((P, 2 * NT), np.float32)
        spack[:, 0:NT] = s.reshape(NT, P).T
        spack[:, NT : 2 * NT] = bias.reshape(NT, P).T
        nbpack = np.ascontiguousarray(np.broadcast_to(n[None, :], (P, N)))

        mm_packs.append((mm_pack, mm2_pack))
        vec_packs.append((spack, nbpack))
    return mm_packs, vec_packs


def kernel(encode, kernel, attn_kernel_self, attn_kernel_neighs):
    from concourse.bass_utils import run_bass_kernel_spmd

    mm_packs, vec_packs = _host_prep(
        encode, kernel, attn_kernel_self, attn_kernel_neighs
    )
    nc = _get_compiled()
    in_maps = [
        {
            "mm_pack": mm_packs[b][0],
            "mm2_pack": mm_packs[b][1],
            "spack": vec_packs[b][0],
            "nbpack": vec_packs[b][1],
        }
        for b in range(B)
    ]
    res = run_bass_kernel_spmd(nc, in_maps, core_ids=list(range(B)))
    return np.stack([res.results[b]["out"] for b in range(B)])
